# revision 2
# baseline (speedup 1.0000x reference)
"""Llama3 attention layer (T=2048, 32 q heads / 8 kv heads, D=128, hidden 4096)
on 8 Trainium2 NeuronCores, tensor-parallel over heads.

Per-core shard: 4 q heads + 1 kv head (w_qkv columns), 512 w_o rows.
Each core computes a full [T, 4096] o_proj partial in bf16; the host sums
the 8 partials in fp32 (the all-reduce of the row-parallel w_o).

Device algorithm (per core), all matmuls in bf16 with fp32 PSUM accumulation:
  1. qkv^T = w_shard^T @ hs^T          -> [768, T]  (c on partitions)
     column-block order v, k, q0..q3 so V transposes + K rope finish early
  2. RoPE on q^T/k^T rows via duplicated cos/sin tables (DVE)
  3. V = transpose(v^T) via PE transposes
  4. S^T[s,t] = k^T.T @ q^T (per head), exp on ACT, 0/1 mask on diag tiles
  5. out^T[d,t] += V[s].T @ P^T[s,t]
     denom: fp32 DVE accumulation of P tiles + one bf16 ones-matmul
  6. out[t,:] = (O^T).T @ w_o_shard    -> bf16 partial, DMA to DRAM
"""
import math

import numpy as np
import ml_dtypes

import bass_rust
import concourse.bass as bass
import concourse.mybir as mybir
import concourse.tile as tile
from concourse.bass_utils import run_bass_kernel_spmd
from concourse.masks import make_identity
from concourse.vector_clock import ScopedClock

BF16 = mybir.dt.bfloat16
F32 = mybir.dt.float32
bf16 = ml_dtypes.bfloat16

T = 2048
HID = 4096
D = 128
NQH = 4          # q heads per core
CB = 6           # qkv col blocks of 128 (v, k, 4 q heads)
HCH = HID // 128  # 32 hidden chunks
TJ = 512          # t tile width
NJ = T // TJ      # 4 t tiles
SB = T // 128     # 16 s blocks
SCALE = 1.0 / math.sqrt(D)

_MAX_CTRL_WAITS = 1


def _install_drain_fix():
    """walrus in this image allows only 1 sem wait on CTRL (nop/drain)
    instructions; spread the Tile tail-drain's global-clock waits across
    preceding sync-engine NOPs."""
    if getattr(tile.TileContext, "_drain_fix_installed", False):
        return

    def _patched(self, tick_clock, wait_clock):
        nc = self.nc
        nops = [nc.sync.nop(nofuse=True, hint=f"drainw{i}") for i in range(32)]
        drain_inst = nc.sync.drain()
        wait_clock.add_sem_waits(
            drain_inst.ins, ScopedClock({None: tick_clock.global_clock})
        )
        si = drain_inst.ins.sync_info
        waits = list(si.on_wait) if si and si.on_wait else []
        if len(waits) > _MAX_CTRL_WAITS:
            chunks = [
                waits[i:i + _MAX_CTRL_WAITS]
                for i in range(0, len(waits), _MAX_CTRL_WAITS)
            ]
            drain_inst.ins.sync_info = bass_rust.SyncInfo(
                on_wait=chunks[-1], on_update=list(si.on_update or [])
            )
            for nop, chunk in zip(nops, chunks[:-1]):
                nop.ins.sync_info = bass_rust.SyncInfo(on_wait=chunk, on_update=[])
        nc.all_engine_barrier()
        assert self.sems is not None
        popped = nc._tile_sem_poison_stack.pop()
        assert popped is self._sem_poison
        nc.clear_and_free_semaphores(list(self.sems.allocated().values()))
        nc.all_engine_barrier()

    tile.TileContext._drain_and_barrier = _patched
    tile.TileContext._drain_fix_installed = True


def _fix_bir_waits(bir_json: bytes, max_waits: int = 1) -> bytes:
    """walrus in this image accepts very few sem waits per instruction.
    Split any instruction carrying more than `max_waits` waits by inserting
    same-engine NoOps ahead of it that carry the excess waits."""
    import json

    bir = json.loads(bir_json)
    n_split = 0
    for fn in bir["functions"]:
        for blk in fn["blocks"]:
            out = []
            for inst in blk["instructions"]:
                si = inst.get("sync_info")
                waits = (si or {}).get("on_wait") or []
                if len(waits) > max_waits:
                    chunks = [
                        waits[i:i + max_waits]
                        for i in range(0, len(waits), max_waits)
                    ]
                    for k, ch in enumerate(chunks[:-1]):
                        out.append(
                            {
                                "debug": inst.get("debug", 0),
                                "engine": inst["engine"],
                                "ins": [],
                                "name": f"{inst['name']}-w{k}",
                                "opcode": "NoOp",
                                "outs": [],
                                "sync_info": {"on_update": [], "on_wait": ch},
                            }
                        )
                        n_split += 1
                    si["on_wait"] = chunks[-1]
                out.append(inst)
            blk["instructions"] = out
    return json.dumps(bir).encode()


def build_nc() -> bass.Bass:
    _install_drain_fix()
    nc = bass.Bass()

    hsT_d = nc.dram_tensor("hsT", [HID, T], BF16, kind="ExternalInput")
    wqkv_d = nc.dram_tensor("wqkv", [HID, CB * 128], BF16, kind="ExternalInput")
    wo_d = nc.dram_tensor("wo", [NQH * 128, HID], BF16, kind="ExternalInput")
    cos_d = nc.dram_tensor("cos2", [128, T], BF16, kind="ExternalInput")
    sin_d = nc.dram_tensor("sin2", [128, T], BF16, kind="ExternalInput")
    mask_d = nc.dram_tensor("masks", [4, 128, TJ], BF16, kind="ExternalInput")
    out_d = nc.dram_tensor("out", [T, HID], BF16, kind="ExternalOutput")

    hsT_r = hsT_d.rearrange("(o p) t -> p o t", p=128)     # [128, 32, T]
    wqkv_r = wqkv_d.rearrange("(o p) c -> p o c", p=128)   # [128, 32, 768]
    wo_r = wo_d.rearrange("(o p) n -> p o n", p=128)       # [128, 4, HID]

    with tile.TileContext(nc) as tc:
        with (
            tc.tile_pool(name="const", bufs=1) as constp,
            tc.tile_pool(name="acts", bufs=1) as actp,
            tc.tile_pool(name="ps", bufs=8, space="PSUM") as psp,
            tc.tile_pool(name="hst", bufs=1) as hstp,
            tc.tile_pool(name="wq", bufs=3) as wqp,
            tc.tile_pool(name="qkt", bufs=3) as qktp,
            tc.tile_pool(name="rtmp", bufs=2) as rtp,
            tc.tile_pool(name="pp", bufs=10) as ppp,
            tc.tile_pool(name="accp", bufs=3) as accp,
            tc.tile_pool(name="rcp", bufs=1) as rcpp,
            tc.tile_pool(name="outp", bufs=4) as outp,
        ):
            cos_sb = constp.tile([128, T], BF16, tag="cos")
            sin_sb = constp.tile([128, T], BF16, tag="sin")
            mask_sb = [
                constp.tile([128, TJ], BF16, tag=f"mask{r}", name=f"mask{r}")
                for r in range(4)
            ]
            ones_sb = constp.tile([128, 128], BF16, tag="ones")
            ident_sb = constp.tile([128, 128], BF16, tag="ident")
            warm_sb = constp.tile([128, TJ], BF16, tag="warm")

            # PE warmup: ~14 dummy matmuls fill the initial DMA wait so the
            # HAM clock-gate opens before real work arrives.
            nc.vector.memset(warm_sb[:], 0.0)
            for wi in range(14):
                pw = psp.tile([128, TJ], F32, tag="ps", name="ps_warm")
                nc.tensor.matmul(
                    pw[:], warm_sb[:, 0:128], warm_sb[:], start=True, stop=True
                )

            # consts on the ACT HWDGE ring (SP ring carries the bulk stream)
            nc.scalar.dma_start(cos_sb[:], cos_d[:])
            nc.scalar.dma_start(sin_sb[:], sin_d[:])
            for r in range(4):
                nc.scalar.dma_start(mask_sb[r][:], mask_d[r])
            nc.vector.memset(ones_sb[:], 1.0)
            make_identity(nc, ident_sb[:])

            # persistent activations
            wo_sb = actp.tile([128, NQH, HID], BF16, tag="wo")
            qkr_sb = [actp.tile([128, T], BF16, tag=f"qkr{c}", name=f"qkr{c}") for c in range(5)]
            vT_sb = actp.tile([128, T], BF16, tag="vT")
            v_sb = [actp.tile([128, 128], BF16, tag=f"v{i}", name=f"v{i}") for i in range(SB)]
            ot_sb = [actp.tile([128, T], BF16, tag=f"ot{h}", name=f"ot{h}") for h in range(NQH)]

            def oproj_tile(t, n):
                t128 = bass.ts(t, 128)
                ps = psp.tile([128, TJ], F32, tag="ps", name="ps_op")
                for c in range(NQH):
                    nc.tensor.matmul(
                        ps[:], ot_sb[c][:, t128], wo_sb[:, c, bass.ts(n, TJ)],
                        start=(c == 0), stop=(c == NQH - 1),
                    )
                ob = outp.tile([128, TJ], BF16, tag="out")
                if n % 2 == 0:
                    nc.scalar.copy(ob[:], ps[:])
                else:
                    nc.vector.tensor_copy(ob[:], ps[:])
                nc.sync.dma_start(out_d[t128, bass.ts(n, TJ)], ob[:])

            for j in range(NJ):
                js = bass.ts(j, TJ)
                # ---- QKV^T for this t tile ----
                hst_j = hstp.tile([128, HCH, TJ], BF16, tag="hst")
                w_tiles = [
                    wqp.tile([128, HCH, 128], BF16, tag="w", name=f"w{j}_{cb}")
                    for cb in range(CB)
                ]
                if j == 0:
                    # interleave first w block with hst chunks so the first
                    # accumulation chain can start as soon as chunk 0 lands
                    for q in range(4):
                        nc.sync.dma_start(
                            w_tiles[0][:, q * 8:(q + 1) * 8, :],
                            wqkv_r[:, q * 8:(q + 1) * 8, 0:128],
                        )
                        nc.sync.dma_start(
                            hst_j[:, q * 8:(q + 1) * 8, :],
                            hsT_r[:, q * 8:(q + 1) * 8, js],
                        )
                else:
                    nc.sync.dma_start(w_tiles[0][:, 0:8, :], wqkv_r[:, 0:8, 0:128])
                    for q in range(4):
                        nc.sync.dma_start(
                            hst_j[:, q * 8:(q + 1) * 8, :],
                            hsT_r[:, q * 8:(q + 1) * 8, js],
                        )
                for cb in range(CB):
                    for q in range(4):
                        if cb == 0 and (q == 0 or j == 0):
                            continue
                        nc.sync.dma_start(
                            w_tiles[cb][:, q * 8:(q + 1) * 8, :],
                            wqkv_r[:, q * 8:(q + 1) * 8, bass.ts(cb, 128)],
                        )
                for cb in range(CB):
                    ps = psp.tile([128, TJ], F32, tag="ps", name="ps_qkv")
                    for h in range(HCH):
                        nc.tensor.matmul(
                            ps[:], w_tiles[cb][:, h, :], hst_j[:, h, :],
                            start=(h == 0), stop=(h == HCH - 1),
                        )
                    if cb == 0:
                        # V block: copy to SBUF then PE-transpose to [s, d]
                        nc.vector.tensor_copy(vT_sb[:, js], ps[:])
                        for i in range(4 * j, 4 * j + 4):
                            pv = psp.tile([128, 128], BF16, tag="ps", name="ps_vt")
                            nc.tensor.transpose(
                                pv[:], vT_sb[:, bass.ts(i, 128)], ident_sb[:]
                            )
                            nc.vector.tensor_copy(v_sb[i][:], pv[:])
                    else:
                        # k (cb=1) -> qkr_sb[4]; q head h (cb=2+h) -> qkr_sb[h]
                        dst = qkr_sb[4] if cb == 1 else qkr_sb[cb - 2]
                        qk_t = qktp.tile([128, TJ], BF16, tag="qkt")
                        nc.scalar.copy(qk_t[:], ps[:])
                        # rope: q' = q*cos2 + swap(q)*sin2 (sin2 top half negated)
                        swp = rtp.tile([128, TJ], BF16, tag="swp")
                        nc.vector.tensor_copy(swp[0:64, :], qk_t[64:128, :])
                        nc.vector.tensor_copy(swp[64:128, :], qk_t[0:64, :])
                        ta = rtp.tile([128, TJ], BF16, tag="ta")
                        nc.vector.tensor_tensor(
                            ta[:], qk_t[:], cos_sb[:, js], mybir.AluOpType.mult
                        )
                        tb = rtp.tile([128, TJ], BF16, tag="tb")
                        nc.vector.tensor_tensor(
                            tb[:], swp[:], sin_sb[:, js], mybir.AluOpType.mult
                        )
                        nc.vector.tensor_tensor(
                            dst[:, js], ta[:], tb[:], mybir.AluOpType.add
                        )
                # ---- attention for this t tile ----
                kT = qkr_sb[4]
                nblk = 4 * j + 4

                for h in range(NQH):
                    qT = qkr_sb[h]
                    ps_o = psp.tile([128, TJ], F32, tag="ps", name="ps_o")
                    acc = accp.tile([128, TJ], F32, tag="acc", name="acc")
                    for i in range(nblk):
                        ps_s = psp.tile([128, TJ], F32, tag="ps", name="ps_s")
                        nc.tensor.matmul(
                            ps_s[:], kT[:, bass.ts(i, 128)], qT[:, js],
                            start=True, stop=True,
                        )
                        p_ij = ppp.tile([128, TJ], BF16, tag="p")
                        nc.scalar.activation(
                            p_ij[:], ps_s[:],
                            mybir.ActivationFunctionType.Exp, scale=SCALE,
                        )
                        if i >= 4 * j:
                            nc.vector.tensor_tensor(
                                p_ij[:], p_ij[:], mask_sb[i - 4 * j][:],
                                mybir.AluOpType.mult,
                            )
                        nc.tensor.matmul(
                            ps_o[:], v_sb[i][:], p_ij[:],
                            start=(i == 0), stop=(i == nblk - 1),
                        )
                        # fp32 running sum of P tiles (softmax denominator)
                        if i == 0:
                            nc.vector.tensor_copy(acc[:], p_ij[:])
                        else:
                            nc.vector.tensor_tensor(
                                acc[:], acc[:], p_ij[:], mybir.AluOpType.add
                            )
                    # den[t] = sum_s acc[s, t] via one bf16 ones-matmul
                    accb = accp.tile([128, TJ], BF16, tag="accb", name="accb")
                    nc.vector.tensor_copy(accb[:], acc[:])
                    ps_den = psp.tile([128, TJ], F32, tag="ps", name="ps_den")
                    nc.tensor.matmul(
                        ps_den[:], ones_sb[:], accb[:], start=True, stop=True
                    )
                    ou = rcpp.tile([128, TJ], F32, tag="ou", name="ou")
                    nc.scalar.copy(ou[:], ps_o[:])
                    rc = rcpp.tile([128, TJ], F32, tag="rc", name="rc")
                    nc.vector.reciprocal(rc[:], ps_den[:])
                    nc.vector.tensor_tensor(
                        ot_sb[h][:, js], ou[:], rc[:], mybir.AluOpType.mult
                    )
                    if j == 0:
                        nc.scalar.dma_start(wo_sb[:, h, :], wo_r[:, h, :])
                # ---- o_proj pipelined one tile behind ----
                if j == 0:
                    continue
                for t in range(4 * (j - 1), 4 * j):
                    for n in range(HID // TJ):
                        oproj_tile(t, n)

            for t in range(4 * (NJ - 1), 4 * NJ):
                for n in range(HID // TJ):
                    oproj_tile(t, n)

    _orig_to_json = nc.to_json_bytes

    def _patched_to_json():
        return _fix_bir_waits(_orig_to_json())

    nc.to_json_bytes = _patched_to_json
    return nc


_NC_CACHE = None


def _get_nc():
    global _NC_CACHE
    if _NC_CACHE is None:
        _NC_CACHE = build_nc()
    return _NC_CACHE


def _host_prep(positions, hidden_states, w_qkv, w_o):
    H, HKV = 32, 8
    pos = np.asarray(positions).astype(np.float32)
    inv_freq = (
        1.0 / (500000.0 ** (np.arange(0, D, 2, dtype=np.float32) / D))
    )
    freqs = pos[:, None] * inv_freq[None, :]                  # [T, 64]
    cos = np.cos(freqs).T                                     # [64, T]
    sin = np.sin(freqs).T
    cos2 = np.concatenate([cos, cos], 0).astype(bf16)         # [128, T]
    sin2 = np.concatenate([-sin, sin], 0).astype(bf16)

    # diagonal 0/1 masks: tile r covers s = 128r+p, t = f (within a 512 tile)
    p = np.arange(128)[:, None]
    f = np.arange(TJ)[None, :]
    masks = np.stack(
        [((128 * r + p) <= f).astype(np.float32) for r in range(4)]
    ).astype(bf16)                                            # [4, 128, 512]

    hsT = np.ascontiguousarray(np.asarray(hidden_states).T).astype(bf16)
    w_qkv = np.asarray(w_qkv)
    w_o = np.asarray(w_o)

    in_maps = []
    for core in range(8):
        qc = slice(core * 4 * D, (core + 1) * 4 * D)
        kc = slice(H * D + core * D, H * D + (core + 1) * D)
        vc = slice((H + HKV) * D + core * D, (H + HKV) * D + (core + 1) * D)
        # column-block order: v, k, q0..q3
        wshard = np.concatenate(
            [w_qkv[:, vc], w_qkv[:, kc], w_qkv[:, qc]], axis=1
        ).astype(bf16)
        woshard = np.ascontiguousarray(
            w_o[core * 512:(core + 1) * 512, :]
        ).astype(bf16)
        in_maps.append(
            {
                "hsT": hsT,
                "wqkv": wshard,
                "wo": woshard,
                "cos2": cos2,
                "sin2": sin2,
                "masks": masks,
            }
        )
    return in_maps


def kernel(positions, hidden_states, w_qkv, w_o, _trace=False):
    nc = _get_nc()
    in_maps = _host_prep(positions, hidden_states, w_qkv, w_o)
    res = run_bass_kernel_spmd(nc, in_maps, list(range(8)), trace=_trace)
    out = np.zeros((T, HID), np.float32)
    for c in range(8):
        out += res.results[c]["out"].astype(np.float32)
    if _trace:
        kernel._last_result = res
    return out


# revision 3
# speedup vs baseline: 1.2515x; 1.2515x over previous
"""Llama3 attention layer (T=2048, 32 q heads / 8 kv heads, D=128, hidden 4096)
on 8 Trainium2 NeuronCores, tensor-parallel over heads.

Per-core shard: 4 q heads + 1 kv head (w_qkv columns), 512 w_o rows.
Each core computes a full [T, 4096] o_proj partial in bf16; the host sums
the 8 partials in fp32 (the all-reduce of the row-parallel w_o).

Device algorithm (per core), all matmuls in bf16 with fp32 PSUM accumulation:
  1. qkv^T = w_shard^T @ hs^T          -> [768, T]  (c on partitions)
     column-block order v, k, q0..q3 so V transposes + K rope finish early
  2. RoPE on q^T/k^T rows via duplicated cos/sin tables (DVE)
  3. V = transpose(v^T) via PE transposes
  4. S^T[s,t] = k^T.T @ q^T (per head), exp on ACT; diagonal s-blocks use
     trapezoid moving slices (width 512-128r) + one [128,128] triangular mask
  5. out^T[d,t] += V[s].T @ P^T[s,t]; denom via ones-matmul; normalize (DVE)
  6. out[t,:] = (O^T).T @ w_o_shard    -> bf16 partial, DMA to DRAM
"""
import math

import numpy as np
import ml_dtypes

import bass_rust
import concourse.bass as bass
import concourse.mybir as mybir
import concourse.tile as tile
from concourse.bass_utils import run_bass_kernel_spmd
from concourse.masks import make_identity
from concourse.vector_clock import ScopedClock

BF16 = mybir.dt.bfloat16
F32 = mybir.dt.float32
bf16 = ml_dtypes.bfloat16

T = 2048
HID = 4096
D = 128
NQH = 4          # q heads per core
CB = 6           # qkv col blocks of 128 (v, k, 4 q heads)
HCH = HID // 128  # 32 hidden chunks
TJ = 512          # t tile width
NJ = T // TJ      # 4 t tiles
SB = T // 128     # 16 s blocks
SCALE = 1.0 / math.sqrt(D)

_MAX_CTRL_WAITS = 1


def _install_drain_fix():
    """walrus in this image allows only 1 sem wait on CTRL (nop/drain)
    instructions; spread the Tile tail-drain's global-clock waits across
    preceding sync-engine NOPs."""
    if getattr(tile.TileContext, "_drain_fix_installed", False):
        return

    def _patched(self, tick_clock, wait_clock):
        nc = self.nc
        nops = [nc.sync.nop(nofuse=True, hint=f"drainw{i}") for i in range(32)]
        drain_inst = nc.sync.drain()
        wait_clock.add_sem_waits(
            drain_inst.ins, ScopedClock({None: tick_clock.global_clock})
        )
        si = drain_inst.ins.sync_info
        waits = list(si.on_wait) if si and si.on_wait else []
        if len(waits) > _MAX_CTRL_WAITS:
            chunks = [
                waits[i:i + _MAX_CTRL_WAITS]
                for i in range(0, len(waits), _MAX_CTRL_WAITS)
            ]
            drain_inst.ins.sync_info = bass_rust.SyncInfo(
                on_wait=chunks[-1], on_update=list(si.on_update or [])
            )
            for nop, chunk in zip(nops, chunks[:-1]):
                nop.ins.sync_info = bass_rust.SyncInfo(on_wait=chunk, on_update=[])
        nc.all_engine_barrier()
        assert self.sems is not None
        popped = nc._tile_sem_poison_stack.pop()
        assert popped is self._sem_poison
        nc.clear_and_free_semaphores(list(self.sems.allocated().values()))
        nc.all_engine_barrier()

    tile.TileContext._drain_and_barrier = _patched
    tile.TileContext._drain_fix_installed = True


def _fix_bir_waits(bir_json: bytes, max_waits: int = 1) -> bytes:
    """walrus in this image accepts very few sem waits per instruction.
    Split any instruction carrying more than `max_waits` waits by inserting
    same-engine NoOps ahead of it that carry the excess waits."""
    import json

    bir = json.loads(bir_json)
    n_split = 0
    for fn in bir["functions"]:
        for blk in fn["blocks"]:
            out = []
            for inst in blk["instructions"]:
                si = inst.get("sync_info")
                waits = (si or {}).get("on_wait") or []
                if len(waits) > max_waits:
                    chunks = [
                        waits[i:i + max_waits]
                        for i in range(0, len(waits), max_waits)
                    ]
                    for k, ch in enumerate(chunks[:-1]):
                        out.append(
                            {
                                "debug": inst.get("debug", 0),
                                "engine": inst["engine"],
                                "ins": [],
                                "name": f"{inst['name']}-w{k}",
                                "opcode": "NoOp",
                                "outs": [],
                                "sync_info": {"on_update": [], "on_wait": ch},
                            }
                        )
                        n_split += 1
                    si["on_wait"] = chunks[-1]
                out.append(inst)
            blk["instructions"] = out
    return json.dumps(bir).encode()


def build_nc() -> bass.Bass:
    _install_drain_fix()
    nc = bass.Bass()

    hsT_d = nc.dram_tensor("hsT", [HID, T], BF16, kind="ExternalInput")
    wqkv_d = nc.dram_tensor("wqkv", [HID, CB * 128], BF16, kind="ExternalInput")
    wo_d = nc.dram_tensor("wo", [NQH * 128, HID], BF16, kind="ExternalInput")
    cos_d = nc.dram_tensor("cos2", [128, T], BF16, kind="ExternalInput")
    sin_d = nc.dram_tensor("sin2", [128, T], BF16, kind="ExternalInput")
    mask_d = nc.dram_tensor("masktri", [128, 128], BF16, kind="ExternalInput")
    out_d = nc.dram_tensor("out", [T, HID], BF16, kind="ExternalOutput")

    hsT_r = hsT_d.rearrange("(o p) t -> p o t", p=128)     # [128, 32, T]
    wqkv_r = wqkv_d.rearrange("(o p) c -> p o c", p=128)   # [128, 32, 768]
    wo_r = wo_d.rearrange("(o p) n -> p o n", p=128)       # [128, 4, HID]

    with tile.TileContext(nc) as tc:
        with (
            tc.tile_pool(name="const", bufs=1) as constp,
            tc.tile_pool(name="acts", bufs=1) as actp,
            tc.tile_pool(name="ps", bufs=8, space="PSUM") as psp,
            tc.tile_pool(name="hst", bufs=1) as hstp,
            tc.tile_pool(name="wq", bufs=2) as wqp,
            tc.tile_pool(name="qkt", bufs=3) as qktp,
            tc.tile_pool(name="rtmp", bufs=2) as rtp,
            tc.tile_pool(name="pp", bufs=10) as ppp,
            tc.tile_pool(name="rcp", bufs=1) as rcpp,
            tc.tile_pool(name="outp", bufs=4) as outp,
        ):
            cos_sb = constp.tile([128, T], BF16, tag="cos")
            sin_sb = constp.tile([128, T], BF16, tag="sin")
            mask_sb = constp.tile([128, 128], BF16, tag="masktri")
            ones_sb = constp.tile([128, 128], BF16, tag="ones")
            ident_sb = constp.tile([128, 128], BF16, tag="ident")
            warm_sb = constp.tile([128, TJ], BF16, tag="warm")

            # PE warmup: dummy matmuls fill the initial DMA wait so the HAM
            # clock-gate opens before real work arrives (~3.4us busy needed).
            nc.vector.memset(warm_sb[:], 0.0)
            for wi in range(26):
                pw = psp.tile([128, TJ], F32, tag="ps", name="ps_warm")
                nc.tensor.matmul(
                    pw[:], warm_sb[:, 0:128], warm_sb[:], start=True, stop=True
                )
            nc.vector.memset(ones_sb[:], 1.0)
            make_identity(nc, ident_sb[:])

            def load_consts():
                nc.sync.dma_start(cos_sb[:], cos_d[:])
                nc.sync.dma_start(sin_sb[:], sin_d[:])
                nc.sync.dma_start(mask_sb[:], mask_d[:])

            # persistent activations
            wo_sb = actp.tile([128, NQH, HID], BF16, tag="wo")
            qkr_sb = [actp.tile([128, T], BF16, tag=f"qkr{c}", name=f"qkr{c}") for c in range(5)]
            vT_sb = actp.tile([128, T], BF16, tag="vT")
            v_sb = [actp.tile([128, 128], BF16, tag=f"v{i}", name=f"v{i}") for i in range(SB)]
            ot_sb = [actp.tile([128, T], BF16, tag=f"ot{h}", name=f"ot{h}") for h in range(NQH)]

            def oproj_tile(t, n):
                t128 = bass.ts(t, 128)
                ps = psp.tile([128, TJ], F32, tag="ps", name="ps_op")
                for c in range(NQH):
                    nc.tensor.matmul(
                        ps[:], ot_sb[c][:, t128], wo_sb[:, c, bass.ts(n, TJ)],
                        start=(c == 0), stop=(c == NQH - 1),
                    )
                ob = outp.tile([128, TJ], BF16, tag="out")
                if n % 2 == 0:
                    nc.scalar.copy(ob[:], ps[:])
                else:
                    nc.vector.tensor_copy(ob[:], ps[:])
                nc.sync.dma_start(out_d[t128, bass.ts(n, TJ)], ob[:])

            for j in range(NJ):
                js = bass.ts(j, TJ)
                # ---- QKV^T for this t tile ----
                # w in two triple-blocks [128, 32, 384]: 768B DMA lines
                hst_j = hstp.tile([128, HCH, TJ], BF16, tag="hst")
                w_tiles = [
                    wqp.tile([128, HCH, 384], BF16, tag="w", name=f"w{j}_{tb}")
                    for tb in range(2)
                ]
                if j == 0:
                    # interleave first w triple with hst chunks so the first
                    # accumulation chain can start as soon as chunk 0 lands
                    for q in range(4):
                        nc.sync.dma_start(
                            w_tiles[0][:, q * 8:(q + 1) * 8, :],
                            wqkv_r[:, q * 8:(q + 1) * 8, 0:384],
                        )
                        nc.sync.dma_start(
                            hst_j[:, q * 8:(q + 1) * 8, :],
                            hsT_r[:, q * 8:(q + 1) * 8, js],
                        )
                else:
                    for q in range(4):
                        nc.sync.dma_start(
                            hst_j[:, q * 8:(q + 1) * 8, :],
                            hsT_r[:, q * 8:(q + 1) * 8, js],
                        )
                    for q in range(4):
                        nc.sync.dma_start(
                            w_tiles[0][:, q * 8:(q + 1) * 8, :],
                            wqkv_r[:, q * 8:(q + 1) * 8, 0:384],
                        )
                for q in range(4):
                    nc.sync.dma_start(
                        w_tiles[1][:, q * 8:(q + 1) * 8, :],
                        wqkv_r[:, q * 8:(q + 1) * 8, 384:768],
                    )
                for cb in range(CB):
                    wt = w_tiles[cb // 3][:, :, bass.ts(cb % 3, 128)]
                    ps = psp.tile([128, TJ], F32, tag="ps", name="ps_qkv")
                    for h in range(HCH):
                        nc.tensor.matmul(
                            ps[:], wt[:, h, :], hst_j[:, h, :],
                            start=(h == 0), stop=(h == HCH - 1),
                        )
                    if j == 0 and cb == 0:
                        load_consts()
                    if cb == 0:
                        # V block: copy to SBUF then PE-transpose to [s, d]
                        nc.vector.tensor_copy(vT_sb[:, js], ps[:])
                        for i in range(4 * j, 4 * j + 4):
                            pv = psp.tile([128, 128], BF16, tag="ps", name="ps_vt")
                            nc.tensor.transpose(
                                pv[:], vT_sb[:, bass.ts(i, 128)], ident_sb[:]
                            )
                            nc.vector.tensor_copy(v_sb[i][:], pv[:])
                    else:
                        # k (cb=1) -> qkr_sb[4]; q head h (cb=2+h) -> qkr_sb[h]
                        dst = qkr_sb[4] if cb == 1 else qkr_sb[cb - 2]
                        qk_t = qktp.tile([128, TJ], BF16, tag="qkt")
                        nc.scalar.copy(qk_t[:], ps[:])
                        # rope: q' = q*cos2 + swap(q)*sin2 (sin2 top half negated)
                        swp = rtp.tile([128, TJ], BF16, tag="swp")
                        nc.vector.tensor_copy(swp[0:64, :], qk_t[64:128, :])
                        nc.vector.tensor_copy(swp[64:128, :], qk_t[0:64, :])
                        ta = rtp.tile([128, TJ], BF16, tag="ta")
                        nc.vector.tensor_tensor(
                            ta[:], qk_t[:], cos_sb[:, js], mybir.AluOpType.mult
                        )
                        tb = rtp.tile([128, TJ], BF16, tag="tb")
                        nc.vector.tensor_tensor(
                            tb[:], swp[:], sin_sb[:, js], mybir.AluOpType.mult
                        )
                        nc.vector.tensor_tensor(
                            dst[:, js], ta[:], tb[:], mybir.AluOpType.add
                        )
                # ---- attention for this t tile ----
                kT = qkr_sb[4]
                nblk = 4 * j + 4

                def normalize(h, ps_o, ps_den):
                    ou = rcpp.tile([128, TJ], F32, tag="ou", name="ou")
                    nc.scalar.copy(ou[:], ps_o[:])
                    den = rcpp.tile([128, TJ], F32, tag="den", name="den")
                    nc.vector.tensor_copy(den[:], ps_den[:])
                    rc = rcpp.tile([128, TJ], F32, tag="rc", name="rc")
                    nc.vector.reciprocal(rc[:], den[:])
                    nc.vector.tensor_tensor(
                        ot_sb[h][:, js], ou[:], rc[:], mybir.AluOpType.mult
                    )

                for h in range(NQH):
                    qT = qkr_sb[h]
                    ps_o = psp.tile([128, TJ], F32, tag="ps", name="ps_o")
                    ps_den = psp.tile([128, TJ], F32, tag="ps", name="ps_den")
                    for i in range(nblk):
                        diag = i >= 4 * j
                        r = i - 4 * j
                        # trapezoid: diagonal s-block r only feeds t >= 128r
                        w = TJ - 128 * r if diag else TJ
                        toff = TJ - w
                        tsl = bass.ds(j * TJ + toff, w)
                        osl = bass.ds(toff, w)
                        ps_s = psp.tile([128, TJ], F32, tag="ps", name="ps_s")
                        nc.tensor.matmul(
                            ps_s[:, 0:w], kT[:, bass.ts(i, 128)], qT[:, tsl],
                            start=True, stop=True,
                        )
                        p_ij = ppp.tile([128, TJ], BF16, tag="p")
                        nc.scalar.activation(
                            p_ij[:, 0:w], ps_s[:, 0:w],
                            mybir.ActivationFunctionType.Exp, scale=SCALE,
                        )
                        if diag:
                            # triangular mask on the first 128 cols (s==t block)
                            nc.vector.tensor_tensor(
                                p_ij[:, 0:128], p_ij[:, 0:128], mask_sb[:],
                                mybir.AluOpType.mult,
                            )
                        nc.tensor.matmul(
                            ps_o[:, osl], v_sb[i][:], p_ij[:, 0:w],
                            start=(i == 0), stop=(i == nblk - 1),
                        )
                        nc.tensor.matmul(
                            ps_den[:, osl], ones_sb[:], p_ij[:, 0:w],
                            start=(i == 0), stop=(i == nblk - 1),
                        )
                    normalize(h, ps_o, ps_den)
                    if j == 0:
                        nc.sync.dma_start(wo_sb[:, h, :], wo_r[:, h, :])
                # ---- o_proj pipelined one tile behind ----
                if j == 0:
                    continue
                for t in range(4 * (j - 1), 4 * j):
                    for n in range(HID // TJ):
                        oproj_tile(t, n)

            for t in range(4 * (NJ - 1), 4 * NJ):
                for n in range(HID // TJ):
                    oproj_tile(t, n)

    _orig_to_json = nc.to_json_bytes

    def _patched_to_json():
        return _fix_bir_waits(_orig_to_json())

    nc.to_json_bytes = _patched_to_json
    return nc


_NC_CACHE = None


def _get_nc():
    global _NC_CACHE
    if _NC_CACHE is None:
        _NC_CACHE = build_nc()
    return _NC_CACHE


def _host_prep(positions, hidden_states, w_qkv, w_o):
    H, HKV = 32, 8
    pos = np.asarray(positions).astype(np.float32)
    inv_freq = (
        1.0 / (500000.0 ** (np.arange(0, D, 2, dtype=np.float32) / D))
    )
    freqs = pos[:, None] * inv_freq[None, :]                  # [T, 64]
    cos = np.cos(freqs).T                                     # [64, T]
    sin = np.sin(freqs).T
    cos2 = np.concatenate([cos, cos], 0).astype(bf16)         # [128, T]
    sin2 = np.concatenate([-sin, sin], 0).astype(bf16)

    # triangular 0/1 mask for the s==t diagonal 128-block: keep s <= t
    p = np.arange(128)[:, None]
    f = np.arange(128)[None, :]
    masktri = (p <= f).astype(np.float32).astype(bf16)        # [128, 128]

    hsT = np.ascontiguousarray(np.asarray(hidden_states).T).astype(bf16)
    w_qkv = np.asarray(w_qkv)
    w_o = np.asarray(w_o)

    in_maps = []
    for core in range(8):
        qc = slice(core * 4 * D, (core + 1) * 4 * D)
        kc = slice(H * D + core * D, H * D + (core + 1) * D)
        vc = slice((H + HKV) * D + core * D, (H + HKV) * D + (core + 1) * D)
        # column-block order: v, k, q0..q3
        wshard = np.concatenate(
            [w_qkv[:, vc], w_qkv[:, kc], w_qkv[:, qc]], axis=1
        ).astype(bf16)
        woshard = np.ascontiguousarray(
            w_o[core * 512:(core + 1) * 512, :]
        ).astype(bf16)
        in_maps.append(
            {
                "hsT": hsT,
                "wqkv": wshard,
                "wo": woshard,
                "cos2": cos2,
                "sin2": sin2,
                "masktri": masktri,
            }
        )
    return in_maps


def kernel(positions, hidden_states, w_qkv, w_o, _trace=False):
    nc = _get_nc()
    in_maps = _host_prep(positions, hidden_states, w_qkv, w_o)
    res = run_bass_kernel_spmd(nc, in_maps, list(range(8)), trace=_trace)
    out = np.zeros((T, HID), np.float32)
    for c in range(8):
        out += res.results[c]["out"].astype(np.float32)
    if _trace:
        kernel._last_result = res
    return out
